# revision 9
# baseline (speedup 1.0000x reference)
"""BoxQueryAndGroup Trainium2 kernel (8 NeuronCores, SPMD).

Problem: for each (batch, query-box) pair, the reference multiplies the
in-box mask by arange(N) and takes the first 32 entries of a stable
ascending argsort. Because out-of-box points (and point 0) map to value 0,
that selects exactly the first 32 indices j with (j == 0 or point j outside
the box), in ascending order. local_group_mask is identically False (index 0
is always the first selected element and position 0 is forced False).

Kernel strategy (per core; core c handles batch c//2, query half c%2, so
128 (b,q) pairs per core, one per SBUF partition):
  - Box test on a K=128 point window -> z[q,j] (1 = selected candidate).
  - v = z * (K - j); 4 rounds of DVE max8/max_index/match_replace extract
    the 32 largest v per query = the first 32 candidate indices, ascending.
    Each round's 8 indices (all queries) form one 1024-sample gather batch,
    so gathers stream while later rounds still select.
  - Per round: indices flatten to one bf16 row by DMA, broadcast across
    partitions by a K=1 bf16 PE matmul (exact, values < 128), and become an
    exact one-hot via is_equal against the partition id.
  - The gather itself is an fp32 PE matmul: rows.T @ onehot, where rows is
    a host-packed [K, 128] array with xyz in cols 0..2 and features in cols
    16..79. fp32 weights go through the PE's exact hi/lo bit-split, so the
    selected values are reproduced bit-exactly (verified on HW). Dummy bf16
    matmuls during the prologue pre-warm the PE clock.
  - ScalarE evacuates PSUM; GpSimd subtracts box centers from the xyz rows;
    grouped_xyz is sliced out of new_features on the host (identical rows).
The window has enormous statistical headroom (32nd candidate at index <= 38
for the reference distribution). Each core also emits per-query candidate
counts within the window; any query with count < 32 (never in practice) is
recomputed exactly on the host.
"""
import sys
import numpy as np

for _p in ("/opt/trn_rl_repo", "/root/.axon_site/_ro/trn_rl_repo"):
    if _p not in sys.path:
        sys.path.insert(0, _p)

import concourse.bass as bass
import concourse.mybir as mybir
import concourse.tile as tile
import concourse.bacc as bacc
from concourse.bass_utils import run_bass_kernel_spmd

f32 = mybir.dt.float32
u32 = mybir.dt.uint32
Alu = mybir.AluOpType

B, N, C, NQ, NS = 4, 16384, 64, 256, 32
K = 128          # point scan window
NQ_CORE = 128    # (b,q) pairs per core
NSAMP_TOT = NQ_CORE * NS  # 4096
RB = NQ_CORE * 8          # samples per round batch (all queries x 8 ranks)
N_CORES = 8
N_WARMUP_MM = 18          # PE clock pre-warm matmuls


def _build(tc, outs, ins):
    nc = tc.nc
    nf_d, counts_d = outs
    xk_d, rows_d, qb_d, qbT3_d = ins
    bf16 = mybir.dt.bfloat16

    with tc.tile_pool(name="main", bufs=1) as pool, \
         tc.tile_pool(name="psum", bufs=1, space="PSUM") as psum:
        # ---- loads (spread across DGE queues) ----
        X = pool.tile([128, 3 * K], f32)
        nc.sync.dma_start(X[:, :], xk_d.partition_broadcast(128))

        qb = pool.tile([128, 6], f32)
        nc.sync.dma_start(qb[:, :], qb_d[:, :])

        rows = pool.tile([K, 128], f32)
        nc.scalar.dma_start(rows[:, :], rows_d[:, :])

        c_rep = pool.tile([3, RB], f32)
        nc.scalar.dma_start(c_rep[:, :], qbT3_d[:, :])

        vdesc = pool.tile([128, K], f32)
        nc.gpsimd.iota(vdesc[:, :], [[-1, K]], base=K, channel_multiplier=0,
                       allow_small_or_imprecise_dtypes=True)

        iotap = pool.tile([128, 1], f32)
        nc.gpsimd.iota(iotap[:, :], [[0, 1]], base=0, channel_multiplier=1,
                       allow_small_or_imprecise_dtypes=True)

        ones = pool.tile([1, 128], bf16)
        nc.vector.memset(ones[:, :], 1.0)

        # PE clock pre-warm: dependency-free dummy bf16 matmuls
        wz = pool.tile([128, 512], bf16)
        nc.gpsimd.memset(wz[:, :], 0.0)
        with tc.tile_pool(name="warm", bufs=1, space="PSUM") as warmpool:
            psW = warmpool.tile([128, 512], f32)
            for _ in range(N_WARMUP_MM):
                nc.tensor.matmul(psW[:, :], wz[:, 0:128], wz[:, :])

        # ---- per-query scalars ----
        h3 = pool.tile([128, 3], f32)
        nc.vector.tensor_scalar(h3[:, :], qb[:, 3:6], 0.5, None, Alu.mult)
        nc3 = pool.tile([128, 3], f32)
        nc.vector.tensor_scalar(nc3[:, :], qb[:, 0:3], -1.0, None, Alu.mult)

        def Xd(d):
            return X[:, :].rearrange("p (k d) -> p d k", d=3)[:, d, :]

        # ---- candidate mask: z = 1 iff outside box (z[:,0] forced 1) ----
        # a_d = |x_d - c_d| on ScalarE (abs LUT, bias = -c), then fold the
        # three "<= h_d" tests on DVE.
        Af = mybir.ActivationFunctionType
        a0 = pool.tile([128, K], f32)
        nc.scalar.activation(a0[:, :], Xd(0), Af.Abs, bias=nc3[:, 0:1])
        a1 = pool.tile([128, K], f32)
        nc.scalar.activation(a1[:, :], Xd(1), Af.Abs, bias=nc3[:, 1:2])
        a2 = pool.tile([128, K], f32)
        nc.scalar.activation(a2[:, :], Xd(2), Af.Abs, bias=nc3[:, 2:3])

        inb = pool.tile([128, K], f32)
        nc.vector.tensor_scalar(inb[:, :], a0[:, :], h3[:, 0:1], None, Alu.is_le)
        nc.vector.scalar_tensor_tensor(
            inb[:, :], a1[:, :], h3[:, 1:2], inb[:, :],
            Alu.is_le, Alu.logical_and,
        )
        nc.vector.scalar_tensor_tensor(
            inb[:, :], a2[:, :], h3[:, 2:3], inb[:, :],
            Alu.is_le, Alu.logical_and,
        )
        z = pool.tile([128, K], f32)
        nc.vector.tensor_scalar(z[:, :], inb[:, :], 0.5, None, Alu.is_lt)
        nc.vector.memset(z[:, 0:1], 1.0)

        counts = pool.tile([128, 1], f32)
        nc.vector.reduce_sum(counts[:, :], z[:, :], axis=mybir.AxisListType.X)
        nc.scalar.dma_start(counts_d[:, :], counts[:, :])

        # ---- first-32 selection: v = z*(K-j); 4x max8 rounds ----
        v = pool.tile([128, K], f32)
        nc.vector.tensor_tensor(v[:, :], z[:, :], vdesc[:, :], Alu.mult)

        offs32 = pool.tile([128, NS], u32)
        idxbf = pool.tile([128, NS], bf16)

        # ---- per-round: select 8 ranks, gather 1024 samples via one-hot mm
        with tc.tile_pool(name="rowp", bufs=2) as rowpool, \
             tc.tile_pool(name="oh", bufs=2) as ohpool, \
             tc.tile_pool(name="psi", bufs=2, space="PSUM") as psti, \
             tc.tile_pool(name="psg", bufs=2, space="PSUM") as pstg, \
             tc.tile_pool(name="ob", bufs=2) as obpool:
            for r in range(4):
                ssl = slice(8 * r, 8 * r + 8)
                mx = pool.tile([128, 8], f32, tag="mx")
                nc.vector.max(mx[:, :], v[:, :])
                nc.vector.max_index(offs32[:, ssl], mx[:, :], v[:, :])
                if r < 3:
                    nc.vector.match_replace(v[:, :], mx[:, :], v[:, :], 0.0)
                nc.vector.tensor_copy(idxbf[:, ssl], offs32[:, ssl])

                idxrow = rowpool.tile([1, RB], bf16)
                nc.sync.dma_start(idxrow[:, :], idxbf[:, ssl])

                psI = psti.tile([128, RB], f32)
                nc.tensor.matmul(psI[:, 0:512], ones[:, :], idxrow[:, 0:512])
                nc.tensor.matmul(psI[:, 512:RB], ones[:, :], idxrow[:, 512:RB])
                oh = ohpool.tile([128, RB], f32)
                nc.vector.tensor_scalar(
                    oh[:, :], psI[:, :], iotap[:, 0:1], None, Alu.is_equal
                )
                psG = pstg.tile([128, RB], f32)
                nc.tensor.matmul(psG[:, 0:512], rows[:, :], oh[:, 0:512])
                nc.tensor.matmul(psG[:, 512:RB], rows[:, :], oh[:, 512:RB])

                fsb = obpool.tile([16 + C, RB], f32, tag="fsb")
                nc.scalar.copy(fsb[:, :], psG[0:16 + C, :])
                sub3 = obpool.tile([3, RB], f32, tag="sub3")
                nc.gpsimd.tensor_tensor(
                    sub3[:, :], fsb[0:3, :], c_rep[:, :], Alu.subtract
                )
                nc.sync.dma_start(nf_d[3:3 + C, :, ssl], fsb[16:16 + C, :])
                nc.scalar.dma_start(nf_d[0:3, :, ssl], sub3[:, :])


_CACHE = {}


def _get_program():
    if "nc" in _CACHE:
        return _CACHE["nc"]
    nc = bacc.Bacc("TRN2", target_bir_lowering=False, debug=False)
    ins = [
        nc.dram_tensor("xk", [3 * K], f32, kind="ExternalInput").ap(),
        nc.dram_tensor("rows", [K, 128], f32, kind="ExternalInput").ap(),
        nc.dram_tensor("qb", [NQ_CORE, 6], f32, kind="ExternalInput").ap(),
        nc.dram_tensor("qbT3", [3, RB], f32, kind="ExternalInput").ap(),
    ]
    outs = [
        nc.dram_tensor("nf", [3 + C, NQ_CORE, NS], f32, kind="ExternalOutput").ap(),
        nc.dram_tensor("counts", [NQ_CORE, 1], f32, kind="ExternalOutput").ap(),
    ]
    with tile.TileContext(nc) as tc:
        _build(tc, outs, ins)
    nc.compile()
    _CACHE["nc"] = nc
    return nc


def _in_maps(kx, kf, qbox):
    per_b = {}
    for b in range(B):
        rows = np.zeros((K, 128), np.float32)
        rows[:, 0:3] = kx[b, :K, :]
        rows[:, 16:16 + C] = kf[b, :, :K].T
        per_b[b] = dict(
            xk=np.ascontiguousarray(kx[b, :K, :].reshape(-1)),
            rows=rows,
        )
    maps = []
    for core in range(N_CORES):
        b, half = core // 2, core % 2
        qs = np.ascontiguousarray(qbox[b, half * NQ_CORE:(half + 1) * NQ_CORE, :])
        m = dict(per_b[b])
        m.update(
            qb=qs,
            qbT3=np.ascontiguousarray(
                np.repeat(qs[:, :3].T[:, :, None], 8, axis=2).reshape(3, -1)),
        )
        maps.append(m)
    return maps


def _host_fix_query(kx_b, kf_b, qbox_bq, nf_bq, gx_bq):
    """Exact recompute of one (b, q) pair on the host (window fallback)."""
    center, size = qbox_bq[:3], qbox_bq[3:]
    off = np.abs(kx_b - center[None, :])
    inb = (off <= size[None, :] * 0.5).all(-1)
    z = ~inb
    z[0] = True
    zi = np.flatnonzero(z)[:NS]
    gxq = kx_b[zi, :].T - center[:, None]
    gx_bq[:] = gxq
    nf_bq[0:3] = gxq
    nf_bq[3:] = kf_b[:, zi]


def kernel(key_xyz, key_features, query_box, _want_timing=False):
    kx = np.ascontiguousarray(np.asarray(key_xyz, dtype=np.float32))
    kf = np.ascontiguousarray(np.asarray(key_features, dtype=np.float32))
    qbox = np.ascontiguousarray(np.asarray(query_box, dtype=np.float32))
    assert kx.shape == (B, N, 3) and kf.shape == (B, C, N) and qbox.shape == (B, NQ, 6)

    nc = _get_program()
    res = run_bass_kernel_spmd(nc, _in_maps(kx, kf, qbox), list(range(N_CORES)))

    new_features = np.empty((B, 3 + C, NQ, NS), np.float32)
    mask = np.zeros((B, NQ, NS), dtype=bool)
    for core in range(N_CORES):
        b, half = core // 2, core % 2
        sl = slice(half * NQ_CORE, (half + 1) * NQ_CORE)
        r = res.results[core]
        new_features[b, :, sl, :] = r["nf"]
        counts = r["counts"][:, 0]
        if (counts < NS).any():
            for q in np.flatnonzero(counts < NS):
                gq = half * NQ_CORE + int(q)
                gx_scratch = np.empty((3, NS), np.float32)
                _host_fix_query(
                    kx[b], kf[b], qbox[b, gq],
                    new_features[b, :, gq, :], gx_scratch,
                )
    grouped_xyz = np.ascontiguousarray(new_features[:, 0:3])
    out = (grouped_xyz, new_features, mask)
    if _want_timing:
        return out, res
    return out


# revision 11
# speedup vs baseline: 1.2922x; 1.2922x over previous
"""BoxQueryAndGroup Trainium2 kernel (8 NeuronCores, SPMD).

Problem: for each (batch, query-box) pair, the reference multiplies the
in-box mask by arange(N) and takes the first 32 entries of a stable
ascending argsort. Because out-of-box points (and point 0) map to value 0,
that selects exactly the first 32 indices j with (j == 0 or point j outside
the box), in ascending order. local_group_mask is identically False (index 0
is always the first selected element and position 0 is forced False).

Kernel strategy (per core; core c handles batch c//2, query half c%2, so
128 (b,q) pairs per core, one per SBUF partition):
  - Box test on a K=128 point window -> z[q,j] (1 = selected candidate).
  - v = z * (K - j); 4 rounds of DVE max8/max_index/match_replace extract
    the 32 largest v per query = the first 32 candidate indices, ascending.
    Each round's 8 indices (all queries) form one 1024-sample gather batch,
    so gathers stream while later rounds still select.
  - Per round: indices flatten to one bf16 row by DMA, are broadcast
    across partitions by K=1 bf16 PE matmuls (exact, values < 128), and an
    is_equal against the partition id turns them into an exact one-hot.
  - The gather itself is an fp32 PE matmul: rows.T @ onehot, where rows is
    a host-packed [K, 128] array with xyz in cols 0..2 and features in cols
    16..79. fp32 weights go through the PE's exact hi/lo bit-split, so the
    selected values are reproduced bit-exactly (verified on HW). Dummy bf16
    matmuls during the prologue pre-warm the PE clock.
  - ScalarE evacuates PSUM; GpSimd subtracts box centers from the xyz rows;
    grouped_xyz is sliced out of new_features on the host (identical rows).
The window has enormous statistical headroom (32nd candidate at index <= 38
for the reference distribution). Each core also emits per-query candidate
counts within the window; any query with count < 32 (never in practice) is
recomputed exactly on the host.
"""
import sys
import numpy as np

for _p in ("/opt/trn_rl_repo", "/root/.axon_site/_ro/trn_rl_repo"):
    if _p not in sys.path:
        sys.path.insert(0, _p)

import concourse.bass as bass
import concourse.mybir as mybir
import concourse.tile as tile
import concourse.bacc as bacc
from concourse.bass_utils import run_bass_kernel_spmd

f32 = mybir.dt.float32
u32 = mybir.dt.uint32
Alu = mybir.AluOpType

B, N, C, NQ, NS = 4, 16384, 64, 256, 32
K = 128          # point scan window
NQ_CORE = 128    # (b,q) pairs per core
NSAMP_TOT = NQ_CORE * NS  # 4096
RB = NQ_CORE * 8          # samples per round batch (all queries x 8 ranks)
N_CORES = 8
N_WARMUP_MM = 26          # PE clock pre-warm matmuls


def _build(tc, outs, ins):
    nc = tc.nc
    nf_d, counts_d = outs
    xk_d, rows_d, qb_d, qbT3_d = ins
    bf16 = mybir.dt.bfloat16

    with tc.tile_pool(name="main", bufs=1) as pool, \
         tc.tile_pool(name="psum", bufs=1, space="PSUM") as psum:
        # ---- loads (spread across DGE queues) ----
        X = pool.tile([128, 3 * K], f32)
        nc.sync.dma_start(X[:, :], xk_d.partition_broadcast(128))

        qb = pool.tile([128, 6], f32)
        nc.sync.dma_start(qb[:, :], qb_d[:, :])

        rows = pool.tile([K, 128], f32)
        nc.gpsimd.dma_start(rows[:, :], rows_d[:, :])

        c_rep = pool.tile([3, RB], f32)
        nc.gpsimd.dma_start(c_rep[:, :], qbT3_d[:, :])

        vdesc = pool.tile([128, K], f32)
        nc.gpsimd.iota(vdesc[:, :], [[-1, K]], base=K, channel_multiplier=0,
                       allow_small_or_imprecise_dtypes=True)

        iotap = pool.tile([128, 1], f32)
        nc.gpsimd.iota(iotap[:, :], [[0, 1]], base=0, channel_multiplier=1,
                       allow_small_or_imprecise_dtypes=True)

        ones = pool.tile([1, 128], bf16)
        nc.vector.memset(ones[:, :], 1.0)

        # PE clock pre-warm: dependency-free dummy bf16 matmuls
        wz = pool.tile([128, 512], bf16)
        nc.gpsimd.memset(wz[:, :], 0.0)
        with tc.tile_pool(name="warm", bufs=1, space="PSUM") as warmpool:
            psW = warmpool.tile([128, 512], f32)
            for _ in range(N_WARMUP_MM):
                nc.tensor.matmul(psW[:, :], wz[:, 0:128], wz[:, :])

        # ---- per-query scalars ----
        h3 = pool.tile([128, 3], f32)
        nc.vector.tensor_scalar(h3[:, :], qb[:, 3:6], 0.5, None, Alu.mult)
        nc3 = pool.tile([128, 3], f32)
        nc.vector.tensor_scalar(nc3[:, :], qb[:, 0:3], -1.0, None, Alu.mult)

        def Xd(d):
            return X[:, :].rearrange("p (k d) -> p d k", d=3)[:, d, :]

        # ---- candidate mask: z = 1 iff outside box (z[:,0] forced 1) ----
        # a_d = |x_d - c_d| on ScalarE (abs LUT, bias = -c), then fold the
        # three "<= h_d" tests on DVE.
        Af = mybir.ActivationFunctionType
        a0 = pool.tile([128, K], f32)
        nc.scalar.activation(a0[:, :], Xd(0), Af.Abs, bias=nc3[:, 0:1])
        a1 = pool.tile([128, K], f32)
        nc.scalar.activation(a1[:, :], Xd(1), Af.Abs, bias=nc3[:, 1:2])
        a2 = pool.tile([128, K], f32)
        nc.scalar.activation(a2[:, :], Xd(2), Af.Abs, bias=nc3[:, 2:3])

        inb = pool.tile([128, K], f32)
        nc.vector.tensor_scalar(inb[:, :], a0[:, :], h3[:, 0:1], None, Alu.is_le)
        nc.vector.scalar_tensor_tensor(
            inb[:, :], a1[:, :], h3[:, 1:2], inb[:, :],
            Alu.is_le, Alu.logical_and,
        )
        nc.vector.scalar_tensor_tensor(
            inb[:, :], a2[:, :], h3[:, 2:3], inb[:, :],
            Alu.is_le, Alu.logical_and,
        )
        z = pool.tile([128, K], f32)
        nc.vector.tensor_scalar(z[:, :], inb[:, :], 0.5, None, Alu.is_lt)
        nc.vector.memset(z[:, 0:1], 1.0)

        counts = pool.tile([128, 1], f32)
        nc.vector.reduce_sum(counts[:, :], z[:, :], axis=mybir.AxisListType.X)

        # ---- first-32 selection: v = z*(K-j); 4x max8 rounds ----
        v = pool.tile([128, K], f32)
        nc.vector.tensor_tensor(v[:, :], z[:, :], vdesc[:, :], Alu.mult)

        offs32 = pool.tile([128, NS], u32)
        idxbf = pool.tile([128, NS], bf16)

        # q-major output accumulators; rounds write strided s-slices
        fsb = pool.tile([16 + C, NSAMP_TOT], f32)
        sub3 = pool.tile([3, NSAMP_TOT], f32)

        def qmajor(t, nch):
            return t[0:nch, :].rearrange("p (q s) -> p q s", s=NS)

        # ---- per-round: select 8 ranks, gather 1024 samples via one-hot mm
        with tc.tile_pool(name="rowp", bufs=2) as rowpool, \
             tc.tile_pool(name="oh", bufs=2) as ohpool, \
             tc.tile_pool(name="psi", bufs=2, space="PSUM") as psti, \
             tc.tile_pool(name="psg", bufs=2, space="PSUM") as pstg:
            for r in range(4):
                ssl = slice(8 * r, 8 * r + 8)
                mx = pool.tile([128, 8], f32, tag="mx")
                nc.vector.max(mx[:, :], v[:, :])
                nc.vector.max_index(offs32[:, ssl], mx[:, :], v[:, :])
                if r < 3:
                    nc.vector.match_replace(v[:, :], mx[:, :], v[:, :], 0.0)
                nc.vector.tensor_copy(idxbf[:, ssl], offs32[:, ssl])

                idxrow = rowpool.tile([1, RB], bf16)
                nc.sync.dma_start(idxrow[:, :], idxbf[:, ssl])

                for hq in range(2):
                    csl = slice(512 * hq, 512 * hq + 512)
                    qs = slice(64 * hq, 64 * hq + 64)
                    psI = psti.tile([128, 512], f32, tag="psI")
                    nc.tensor.matmul(psI[:, :], ones[:, :], idxrow[:, csl])
                    oh = ohpool.tile([128, 512], f32)
                    nc.vector.tensor_scalar(
                        oh[:, :], psI[:, :], iotap[:, 0:1], None, Alu.is_equal
                    )
                    psG = pstg.tile([128, 512], f32, tag="psG")
                    nc.tensor.matmul(psG[:, :], rows[:, :], oh[:, :])
                    nc.scalar.copy(
                        qmajor(fsb, 16 + C)[:, qs, ssl], psG[0:16 + C, :]
                    )
                    nc.gpsimd.tensor_tensor(
                        qmajor(sub3, 3)[:, qs, ssl],
                        qmajor(fsb, 3)[:, qs, ssl],
                        c_rep[:, :].rearrange("p (q s) -> p q s", s=8)[:, qs, :],
                        Alu.subtract,
                    )
        nc.sync.dma_start(nf_d[3:3 + C, :, :], fsb[16:16 + C, :])
        nc.scalar.dma_start(nf_d[0:3, :, :], sub3[:, :])
        nc.scalar.dma_start(counts_d[:, :], counts[:, :])


_CACHE = {}


def _get_program():
    if "nc" in _CACHE:
        return _CACHE["nc"]
    nc = bacc.Bacc("TRN2", target_bir_lowering=False, debug=False)
    ins = [
        nc.dram_tensor("xk", [3 * K], f32, kind="ExternalInput").ap(),
        nc.dram_tensor("rows", [K, 128], f32, kind="ExternalInput").ap(),
        nc.dram_tensor("qb", [NQ_CORE, 6], f32, kind="ExternalInput").ap(),
        nc.dram_tensor("qbT3", [3, RB], f32, kind="ExternalInput").ap(),
    ]
    outs = [
        nc.dram_tensor("nf", [3 + C, NQ_CORE, NS], f32, kind="ExternalOutput").ap(),
        nc.dram_tensor("counts", [NQ_CORE, 1], f32, kind="ExternalOutput").ap(),
    ]
    with tile.TileContext(nc) as tc:
        _build(tc, outs, ins)
    nc.compile()
    _CACHE["nc"] = nc
    return nc


def _in_maps(kx, kf, qbox):
    per_b = {}
    for b in range(B):
        rows = np.zeros((K, 128), np.float32)
        rows[:, 0:3] = kx[b, :K, :]
        rows[:, 16:16 + C] = kf[b, :, :K].T
        per_b[b] = dict(
            xk=np.ascontiguousarray(kx[b, :K, :].reshape(-1)),
            rows=rows,
        )
    maps = []
    for core in range(N_CORES):
        b, half = core // 2, core % 2
        qs = np.ascontiguousarray(qbox[b, half * NQ_CORE:(half + 1) * NQ_CORE, :])
        m = dict(per_b[b])
        m.update(
            qb=qs,
            qbT3=np.ascontiguousarray(
                np.repeat(qs[:, :3].T[:, :, None], 8, axis=2).reshape(3, -1)),
        )
        maps.append(m)
    return maps


def _host_fix_query(kx_b, kf_b, qbox_bq, nf_bq, gx_bq):
    """Exact recompute of one (b, q) pair on the host (window fallback)."""
    center, size = qbox_bq[:3], qbox_bq[3:]
    off = np.abs(kx_b - center[None, :])
    inb = (off <= size[None, :] * 0.5).all(-1)
    z = ~inb
    z[0] = True
    zi = np.flatnonzero(z)[:NS]
    gxq = kx_b[zi, :].T - center[:, None]
    gx_bq[:] = gxq
    nf_bq[0:3] = gxq
    nf_bq[3:] = kf_b[:, zi]


def kernel(key_xyz, key_features, query_box, _want_timing=False):
    kx = np.ascontiguousarray(np.asarray(key_xyz, dtype=np.float32))
    kf = np.ascontiguousarray(np.asarray(key_features, dtype=np.float32))
    qbox = np.ascontiguousarray(np.asarray(query_box, dtype=np.float32))
    assert kx.shape == (B, N, 3) and kf.shape == (B, C, N) and qbox.shape == (B, NQ, 6)

    nc = _get_program()
    res = run_bass_kernel_spmd(nc, _in_maps(kx, kf, qbox), list(range(N_CORES)))

    new_features = np.empty((B, 3 + C, NQ, NS), np.float32)
    mask = np.zeros((B, NQ, NS), dtype=bool)
    for core in range(N_CORES):
        b, half = core // 2, core % 2
        sl = slice(half * NQ_CORE, (half + 1) * NQ_CORE)
        r = res.results[core]
        new_features[b, :, sl, :] = r["nf"]
        counts = r["counts"][:, 0]
        if (counts < NS).any():
            for q in np.flatnonzero(counts < NS):
                gq = half * NQ_CORE + int(q)
                gx_scratch = np.empty((3, NS), np.float32)
                _host_fix_query(
                    kx[b], kf[b], qbox[b, gq],
                    new_features[b, :, gq, :], gx_scratch,
                )
    grouped_xyz = np.ascontiguousarray(new_features[:, 0:3])
    out = (grouped_xyz, new_features, mask)
    if _want_timing:
        return out, res
    return out


# revision 12
# speedup vs baseline: 1.3339x; 1.0323x over previous
"""BoxQueryAndGroup Trainium2 kernel (8 NeuronCores, SPMD).

Problem: for each (batch, query-box) pair, the reference multiplies the
in-box mask by arange(N) and takes the first 32 entries of a stable
ascending argsort. Because out-of-box points (and point 0) map to value 0,
that selects exactly the first 32 indices j with (j == 0 or point j outside
the box), in ascending order. local_group_mask is identically False (index 0
is always the first selected element and position 0 is forced False).

Kernel strategy (per core; core c handles batch c//2, query half c%2, so
128 (b,q) pairs per core, one per SBUF partition):
  - Box test on a K=128 point window -> z[q,j] (1 = selected candidate).
  - v = z * (K - j); 4 rounds of DVE max8/max_index/match_replace extract
    the 32 largest v per query = the first 32 candidate indices, ascending.
    Each round's 8 indices (all queries) form one 1024-sample gather batch,
    so gathers stream while later rounds still select.
  - Per round: indices flatten to one bf16 row by DMA, are broadcast
    across partitions by K=1 bf16 PE matmuls (exact, values < 128), and an
    is_equal against the partition id turns them into an exact one-hot.
  - The gather itself is an fp32 PE matmul: rows.T @ onehot, where rows is
    a host-packed [K, 128] array with xyz in cols 0..2 and features in cols
    16..79. fp32 weights go through the PE's exact hi/lo bit-split, so the
    selected values are reproduced bit-exactly (verified on HW). Dummy bf16
    matmuls during the prologue pre-warm the PE clock.
  - ScalarE evacuates PSUM; GpSimd subtracts box centers from the xyz rows;
    grouped_xyz is sliced out of new_features on the host (identical rows).
The window has enormous statistical headroom (32nd candidate at index <= 38
for the reference distribution). Each core also emits per-query candidate
counts within the window; any query with count < 32 (never in practice) is
recomputed exactly on the host.
"""
import sys
import numpy as np

for _p in ("/opt/trn_rl_repo", "/root/.axon_site/_ro/trn_rl_repo"):
    if _p not in sys.path:
        sys.path.insert(0, _p)

import concourse.bass as bass
import concourse.mybir as mybir
import concourse.tile as tile
import concourse.bacc as bacc
from concourse.bass_utils import run_bass_kernel_spmd

f32 = mybir.dt.float32
u32 = mybir.dt.uint32
Alu = mybir.AluOpType

B, N, C, NQ, NS = 4, 16384, 64, 256, 32
K = 128          # point scan window
NQ_CORE = 128    # (b,q) pairs per core
NSAMP_TOT = NQ_CORE * NS  # 4096
RB = NQ_CORE * 8          # samples per round batch (all queries x 8 ranks)
N_CORES = 8
N_WARMUP_MM = 26          # PE clock pre-warm matmuls


def _build(tc, outs, ins):
    nc = tc.nc
    nf_d, counts_d = outs
    xk_d, rows_d, qb_d, qbT3_d = ins
    bf16 = mybir.dt.bfloat16

    with tc.tile_pool(name="main", bufs=1) as pool, \
         tc.tile_pool(name="psum", bufs=1, space="PSUM") as psum:
        # prefetch the ACT Abs table before real work needs it
        scr = pool.tile([128, 1], f32)
        nc.vector.memset(scr[:, :], 0.0)
        nc.scalar.activation(
            scr[:, :], scr[:, :], mybir.ActivationFunctionType.Abs
        )

        # ---- loads (spread across DGE queues) ----
        X = pool.tile([128, 3 * K], f32)
        nc.sync.dma_start(X[:, :], xk_d.partition_broadcast(128))

        qb = pool.tile([128, 6], f32)
        nc.sync.dma_start(qb[:, :], qb_d[:, :])

        rows = pool.tile([K, 128], f32)
        nc.gpsimd.dma_start(rows[:, :], rows_d[:, :])

        c_rep = pool.tile([3, RB], f32)
        nc.gpsimd.dma_start(c_rep[:, :], qbT3_d[:, :])

        vdesc = pool.tile([128, K], f32)
        nc.gpsimd.iota(vdesc[:, :], [[-1, K]], base=K, channel_multiplier=0,
                       allow_small_or_imprecise_dtypes=True)

        iotap = pool.tile([128, 1], f32)
        nc.gpsimd.iota(iotap[:, :], [[0, 1]], base=0, channel_multiplier=1,
                       allow_small_or_imprecise_dtypes=True)

        ones = pool.tile([1, 128], bf16)
        nc.vector.memset(ones[:, :], 1.0)

        # PE clock pre-warm: dependency-free dummy bf16 matmuls
        wz = pool.tile([128, 512], bf16)
        nc.gpsimd.memset(wz[:, :], 0.0)
        with tc.tile_pool(name="warm", bufs=1, space="PSUM") as warmpool:
            psW = warmpool.tile([128, 512], f32)
            for _ in range(N_WARMUP_MM):
                nc.tensor.matmul(psW[:, :], wz[:, 0:128], wz[:, :])

        # ---- per-query scalars ----
        h3 = pool.tile([128, 3], f32)
        nc.vector.tensor_scalar(h3[:, :], qb[:, 3:6], 0.5, None, Alu.mult)
        nc3 = pool.tile([128, 3], f32)
        nc.vector.tensor_scalar(nc3[:, :], qb[:, 0:3], -1.0, None, Alu.mult)

        def Xd(d):
            return X[:, :].rearrange("p (k d) -> p d k", d=3)[:, d, :]

        # ---- candidate mask: z = 1 iff outside box (z[:,0] forced 1) ----
        # a_d = |x_d - c_d| on ScalarE (abs LUT, bias = -c), then fold the
        # three "<= h_d" tests on DVE.
        Af = mybir.ActivationFunctionType
        a0 = pool.tile([128, K], f32)
        nc.scalar.activation(a0[:, :], Xd(0), Af.Abs, bias=nc3[:, 0:1])
        a1 = pool.tile([128, K], f32)
        nc.scalar.activation(a1[:, :], Xd(1), Af.Abs, bias=nc3[:, 1:2])
        a2 = pool.tile([128, K], f32)
        nc.scalar.activation(a2[:, :], Xd(2), Af.Abs, bias=nc3[:, 2:3])

        inb = pool.tile([128, K], f32)
        nc.vector.tensor_scalar(inb[:, :], a0[:, :], h3[:, 0:1], None, Alu.is_le)
        nc.vector.scalar_tensor_tensor(
            inb[:, :], a1[:, :], h3[:, 1:2], inb[:, :],
            Alu.is_le, Alu.logical_and,
        )
        nc.vector.scalar_tensor_tensor(
            inb[:, :], a2[:, :], h3[:, 2:3], inb[:, :],
            Alu.is_le, Alu.logical_and,
        )
        z = pool.tile([128, K], f32)
        nc.vector.tensor_scalar(z[:, :], inb[:, :], 0.5, None, Alu.is_lt)
        nc.vector.memset(z[:, 0:1], 1.0)

        counts = pool.tile([128, 1], f32)
        nc.vector.reduce_sum(counts[:, :], z[:, :], axis=mybir.AxisListType.X)

        # ---- first-32 selection: v = z*(K-j); 4x max8 rounds ----
        v = pool.tile([128, K], f32)
        nc.vector.tensor_tensor(v[:, :], z[:, :], vdesc[:, :], Alu.mult)

        offs32 = pool.tile([128, NS], u32)
        idxbf = pool.tile([128, NS], bf16)

        # q-major output accumulators; rounds write strided s-slices
        fsb = pool.tile([16 + C, NSAMP_TOT], f32)
        sub3 = pool.tile([3, NSAMP_TOT], f32)

        def qmajor(t, nch):
            return t[0:nch, :].rearrange("p (q s) -> p q s", s=NS)

        # ---- per-round: select 8 ranks, gather 1024 samples via one-hot mm
        with tc.tile_pool(name="rowp", bufs=2) as rowpool, \
             tc.tile_pool(name="oh", bufs=2) as ohpool, \
             tc.tile_pool(name="psi", bufs=2, space="PSUM") as psti, \
             tc.tile_pool(name="psg", bufs=2, space="PSUM") as pstg:
            for r in range(4):
                ssl = slice(8 * r, 8 * r + 8)
                mx = pool.tile([128, 8], f32, tag="mx")
                nc.vector.max(mx[:, :], v[:, :])
                nc.vector.max_index(offs32[:, ssl], mx[:, :], v[:, :])
                if r < 3:
                    nc.vector.match_replace(v[:, :], mx[:, :], v[:, :], 0.0)
                nc.vector.tensor_copy(idxbf[:, ssl], offs32[:, ssl])

                idxrow = rowpool.tile([1, RB], bf16)
                nc.sync.dma_start(idxrow[:, :], idxbf[:, ssl])

                for hq in range(2):
                    csl = slice(512 * hq, 512 * hq + 512)
                    qs = slice(64 * hq, 64 * hq + 64)
                    psI = psti.tile([128, 512], f32, tag="psI")
                    nc.tensor.matmul(psI[:, :], ones[:, :], idxrow[:, csl])
                    oh = ohpool.tile([128, 512], f32)
                    nc.vector.tensor_scalar(
                        oh[:, :], psI[:, :], iotap[:, 0:1], None, Alu.is_equal
                    )
                    psG = pstg.tile([128, 512], f32, tag="psG")
                    nc.tensor.matmul(psG[:, :], rows[:, :], oh[:, :])
                    nc.scalar.copy(
                        qmajor(fsb, 16 + C)[:, qs, ssl], psG[0:16 + C, :]
                    )
                    nc.gpsimd.tensor_tensor(
                        qmajor(sub3, 3)[:, qs, ssl],
                        qmajor(fsb, 3)[:, qs, ssl],
                        c_rep[:, :].rearrange("p (q s) -> p q s", s=8)[:, qs, :],
                        Alu.subtract,
                    )
                    if r == 3:
                        hsl = slice(2048 * hq, 2048 * hq + 2048)
                        nc.sync.dma_start(
                            nf_d[3:3 + C, qs, :], fsb[16:16 + C, hsl]
                        )
                        nc.scalar.dma_start(nf_d[0:3, qs, :], sub3[:, hsl])
        nc.gpsimd.dma_start(counts_d[:, :], counts[:, :])


_CACHE = {}


def _get_program():
    if "nc" in _CACHE:
        return _CACHE["nc"]
    nc = bacc.Bacc("TRN2", target_bir_lowering=False, debug=False)
    ins = [
        nc.dram_tensor("xk", [3 * K], f32, kind="ExternalInput").ap(),
        nc.dram_tensor("rows", [K, 128], f32, kind="ExternalInput").ap(),
        nc.dram_tensor("qb", [NQ_CORE, 6], f32, kind="ExternalInput").ap(),
        nc.dram_tensor("qbT3", [3, RB], f32, kind="ExternalInput").ap(),
    ]
    outs = [
        nc.dram_tensor("nf", [3 + C, NQ_CORE, NS], f32, kind="ExternalOutput").ap(),
        nc.dram_tensor("counts", [NQ_CORE, 1], f32, kind="ExternalOutput").ap(),
    ]
    with tile.TileContext(nc) as tc:
        _build(tc, outs, ins)
    nc.compile()
    _CACHE["nc"] = nc
    return nc


def _in_maps(kx, kf, qbox):
    per_b = {}
    for b in range(B):
        rows = np.zeros((K, 128), np.float32)
        rows[:, 0:3] = kx[b, :K, :]
        rows[:, 16:16 + C] = kf[b, :, :K].T
        per_b[b] = dict(
            xk=np.ascontiguousarray(kx[b, :K, :].reshape(-1)),
            rows=rows,
        )
    maps = []
    for core in range(N_CORES):
        b, half = core // 2, core % 2
        qs = np.ascontiguousarray(qbox[b, half * NQ_CORE:(half + 1) * NQ_CORE, :])
        m = dict(per_b[b])
        m.update(
            qb=qs,
            qbT3=np.ascontiguousarray(
                np.repeat(qs[:, :3].T[:, :, None], 8, axis=2).reshape(3, -1)),
        )
        maps.append(m)
    return maps


def _host_fix_query(kx_b, kf_b, qbox_bq, nf_bq, gx_bq):
    """Exact recompute of one (b, q) pair on the host (window fallback)."""
    center, size = qbox_bq[:3], qbox_bq[3:]
    off = np.abs(kx_b - center[None, :])
    inb = (off <= size[None, :] * 0.5).all(-1)
    z = ~inb
    z[0] = True
    zi = np.flatnonzero(z)[:NS]
    gxq = kx_b[zi, :].T - center[:, None]
    gx_bq[:] = gxq
    nf_bq[0:3] = gxq
    nf_bq[3:] = kf_b[:, zi]


def kernel(key_xyz, key_features, query_box, _want_timing=False):
    kx = np.ascontiguousarray(np.asarray(key_xyz, dtype=np.float32))
    kf = np.ascontiguousarray(np.asarray(key_features, dtype=np.float32))
    qbox = np.ascontiguousarray(np.asarray(query_box, dtype=np.float32))
    assert kx.shape == (B, N, 3) and kf.shape == (B, C, N) and qbox.shape == (B, NQ, 6)

    nc = _get_program()
    res = run_bass_kernel_spmd(nc, _in_maps(kx, kf, qbox), list(range(N_CORES)))

    new_features = np.empty((B, 3 + C, NQ, NS), np.float32)
    mask = np.zeros((B, NQ, NS), dtype=bool)
    for core in range(N_CORES):
        b, half = core // 2, core % 2
        sl = slice(half * NQ_CORE, (half + 1) * NQ_CORE)
        r = res.results[core]
        new_features[b, :, sl, :] = r["nf"]
        counts = r["counts"][:, 0]
        if (counts < NS).any():
            for q in np.flatnonzero(counts < NS):
                gq = half * NQ_CORE + int(q)
                gx_scratch = np.empty((3, NS), np.float32)
                _host_fix_query(
                    kx[b], kf[b], qbox[b, gq],
                    new_features[b, :, gq, :], gx_scratch,
                )
    grouped_xyz = np.ascontiguousarray(new_features[:, 0:3])
    out = (grouped_xyz, new_features, mask)
    if _want_timing:
        return out, res
    return out


# revision 13
# speedup vs baseline: 1.6855x; 1.2637x over previous
"""BoxQueryAndGroup Trainium2 kernel (8 NeuronCores, SPMD).

Problem: for each (batch, query-box) pair, the reference multiplies the
in-box mask by arange(N) and takes the first 32 entries of a stable
ascending argsort. Because out-of-box points (and point 0) map to value 0,
that selects exactly the first 32 indices j with (j == 0 or point j outside
the box), in ascending order. local_group_mask is identically False (index 0
is always the first selected element and position 0 is forced False).

Kernel strategy (per core; core c handles batch c//2, query half c%2, so
128 (b,q) pairs per core, one per SBUF partition):
  - Box test on a K=128 point window -> z[q,j] (1 = selected candidate).
  - v = z * (K - j); 4 rounds of DVE max8/max_index/match_replace extract
    the 32 largest v per query = the first 32 candidate indices, ascending.
    Each round's 8 indices (all queries) form one 1024-sample gather batch,
    so gathers stream while later rounds still select.
  - Per round: indices flatten to one bf16 row by DMA, are broadcast
    across partitions by K=1 bf16 PE matmuls (exact, values < 128), and an
    is_equal against the partition id turns them into an exact one-hot.
  - The gather itself is an fp32 PE matmul: rows.T @ onehot, where rows is
    a host-packed [K, 128] array with xyz in cols 0..2 and features in cols
    16..79. fp32 weights go through the PE's exact hi/lo bit-split, so the
    selected values are reproduced bit-exactly (verified on HW). Dummy bf16
    matmuls during the prologue pre-warm the PE clock.
  - ScalarE evacuates PSUM; GpSimd subtracts box centers from the xyz rows;
    grouped_xyz is sliced out of new_features on the host (identical rows).
The window has enormous statistical headroom (32nd candidate at index <= 38
for the reference distribution). Each core also emits the 32nd-largest
selection value per query; a zero there means the window held fewer than 32
candidates (never in practice) and that query is recomputed on the host.
"""
import sys
import numpy as np

for _p in ("/opt/trn_rl_repo", "/root/.axon_site/_ro/trn_rl_repo"):
    if _p not in sys.path:
        sys.path.insert(0, _p)

import concourse.bass as bass
import concourse.mybir as mybir
import concourse.tile as tile
import concourse.bacc as bacc
from concourse.bass_utils import run_bass_kernel_spmd

f32 = mybir.dt.float32
u32 = mybir.dt.uint32
Alu = mybir.AluOpType

B, N, C, NQ, NS = 4, 16384, 64, 256, 32
K = 128          # point scan window
NQ_CORE = 128    # (b,q) pairs per core
NSAMP_TOT = NQ_CORE * NS  # 4096
RB = NQ_CORE * 8          # samples per round batch (all queries x 8 ranks)
N_CORES = 8
N_WARMUP_MM = 22          # PE clock pre-warm matmuls


def _build(tc, outs, ins):
    nc = tc.nc
    nf_d, counts_d = outs
    xk_d, rows_d, qb_d, qbT3_d = ins
    bf16 = mybir.dt.bfloat16

    with tc.tile_pool(name="main", bufs=1) as pool, \
         tc.tile_pool(name="psum", bufs=1, space="PSUM") as psum:
        # prefetch the ACT Abs table before real work needs it
        scr = pool.tile([128, 1], f32)
        nc.vector.memset(scr[:, :], 0.0)
        nc.scalar.activation(
            scr[:, :], scr[:, :], mybir.ActivationFunctionType.Abs
        )

        # ---- loads (spread across DGE queues) ----
        xrow = pool.tile([1, 3 * K], f32)
        nc.sync.dma_start(xrow[:, :], xk_d.unsqueeze(0))

        onesf = pool.tile([1, 128], f32)
        nc.vector.memset(onesf[:, :], 1.0)
        psX = psum.tile([128, 3 * K], f32)
        nc.tensor.matmul(psX[:, :], onesf[:, :], xrow[:, :])

        qb = pool.tile([128, 6], f32)
        nc.sync.dma_start(qb[:, :], qb_d[:, :])

        rows = pool.tile([K, 128], f32)
        nc.gpsimd.dma_start(rows[:, :], rows_d[:, :])

        c_rep = pool.tile([3, RB], f32)
        nc.gpsimd.dma_start(c_rep[:, :], qbT3_d[:, :])

        vdesc = pool.tile([128, K], f32)
        nc.gpsimd.iota(vdesc[:, :], [[-1, K]], base=K, channel_multiplier=0,
                       allow_small_or_imprecise_dtypes=True)

        iotap = pool.tile([128, 1], f32)
        nc.gpsimd.iota(iotap[:, :], [[0, 1]], base=0, channel_multiplier=1,
                       allow_small_or_imprecise_dtypes=True)

        ones = pool.tile([1, 128], bf16)
        nc.vector.memset(ones[:, :], 1.0)

        # PE clock pre-warm: dependency-free dummy bf16 matmuls
        wz = pool.tile([128, 512], bf16)
        nc.vector.memset(wz[:, :], 0.0)
        with tc.tile_pool(name="warm", bufs=1, space="PSUM") as warmpool:
            psW = warmpool.tile([128, 512], f32)
            for _ in range(N_WARMUP_MM):
                nc.tensor.matmul(psW[:, :], wz[:, 0:128], wz[:, :])

        # ---- per-query scalars ----
        h3 = pool.tile([128, 3], f32)
        nc.vector.tensor_scalar(h3[:, :], qb[:, 3:6], 0.5, None, Alu.mult)
        nc3 = pool.tile([128, 3], f32)
        nc.vector.tensor_scalar(nc3[:, :], qb[:, 0:3], -1.0, None, Alu.mult)

        def Xd(d):
            return psX[:, :].rearrange("p (k d) -> p d k", d=3)[:, d, :]

        # ---- candidate mask: z = 1 iff outside box (z[:,0] forced 1) ----
        # a_d = |x_d - c_d| on ScalarE (abs LUT, bias = -c), then fold the
        # three "<= h_d" tests on DVE.
        Af = mybir.ActivationFunctionType
        a0 = pool.tile([128, K], f32)
        nc.scalar.activation(a0[:, :], Xd(0), Af.Abs, bias=nc3[:, 0:1])
        a1 = pool.tile([128, K], f32)
        nc.scalar.activation(a1[:, :], Xd(1), Af.Abs, bias=nc3[:, 1:2])
        a2 = pool.tile([128, K], f32)
        nc.scalar.activation(a2[:, :], Xd(2), Af.Abs, bias=nc3[:, 2:3])

        inb = pool.tile([128, K], f32)
        nc.vector.tensor_scalar(inb[:, :], a0[:, :], h3[:, 0:1], None, Alu.is_le)
        nc.vector.scalar_tensor_tensor(
            inb[:, :], a1[:, :], h3[:, 1:2], inb[:, :],
            Alu.is_le, Alu.logical_and,
        )
        nc.vector.scalar_tensor_tensor(
            inb[:, :], a2[:, :], h3[:, 2:3], inb[:, :],
            Alu.is_le, Alu.logical_and,
        )
        z = pool.tile([128, K], f32)
        nc.vector.tensor_scalar(z[:, :], inb[:, :], 0.5, None, Alu.is_lt)
        nc.vector.memset(z[:, 0:1], 1.0)

        # ---- first-32 selection: v = z*(K-j); 4x max8 rounds ----
        v = pool.tile([128, K], f32)
        nc.vector.tensor_tensor(v[:, :], z[:, :], vdesc[:, :], Alu.mult)

        offs32 = pool.tile([128, NS], u32)
        idxbf = pool.tile([128, NS], bf16)

        # q-major output accumulators; rounds write strided s-slices
        fsb = pool.tile([16 + C, NSAMP_TOT], f32)
        sub3 = pool.tile([3, NSAMP_TOT], f32)

        def qmajor(t, nch):
            return t[0:nch, :].rearrange("p (q s) -> p q s", s=NS)

        # ---- per-round: select 8 ranks, gather 1024 samples via one-hot mm
        with tc.tile_pool(name="rowp", bufs=2) as rowpool, \
             tc.tile_pool(name="oh", bufs=2) as ohpool, \
             tc.tile_pool(name="psi", bufs=2, space="PSUM") as psti, \
             tc.tile_pool(name="psg", bufs=2, space="PSUM") as pstg:
            for r in range(4):
                ssl = slice(8 * r, 8 * r + 8)
                mx = pool.tile([128, 8], f32, tag=f"mx{r}")
                nc.vector.max(mx[:, :], v[:, :])
                if r == 3:
                    mx3 = mx
                nc.vector.max_index(offs32[:, ssl], mx[:, :], v[:, :])
                if r < 3:
                    nc.vector.match_replace(v[:, :], mx[:, :], v[:, :], 0.0)
                nc.vector.tensor_copy(idxbf[:, ssl], offs32[:, ssl])

                idxrow = rowpool.tile([1, RB], bf16)
                nc.sync.dma_start(idxrow[:, :], idxbf[:, ssl])

                for hq in range(2):
                    csl = slice(512 * hq, 512 * hq + 512)
                    qs = slice(64 * hq, 64 * hq + 64)
                    psI = psti.tile([128, 512], f32, tag="psI")
                    nc.tensor.matmul(psI[:, :], ones[:, :], idxrow[:, csl])
                    oh = ohpool.tile([128, 512], f32)
                    nc.vector.tensor_scalar(
                        oh[:, :], psI[:, :], iotap[:, 0:1], None, Alu.is_equal
                    )
                    psG = pstg.tile([128, 512], f32, tag="psG")
                    nc.tensor.matmul(psG[:, :], rows[:, :], oh[:, :])
                    nc.scalar.copy(
                        qmajor(fsb, 16 + C)[:, qs, ssl], psG[0:16 + C, :]
                    )
                    nc.gpsimd.tensor_tensor(
                        qmajor(sub3, 3)[:, qs, ssl],
                        qmajor(fsb, 3)[:, qs, ssl],
                        c_rep[:, :].rearrange("p (q s) -> p q s", s=8)[:, qs, :],
                        Alu.subtract,
                    )
                    if r == 3:
                        hsl = slice(2048 * hq, 2048 * hq + 2048)
                        nc.sync.dma_start(
                            nf_d[3:3 + C, qs, :], fsb[16:16 + C, hsl]
                        )
                        nc.scalar.dma_start(nf_d[0:3, qs, :], sub3[:, hsl])
        nc.gpsimd.dma_start(counts_d[:, :], mx3[:, 7:8])


_CACHE = {}


def _get_program():
    if "nc" in _CACHE:
        return _CACHE["nc"]
    nc = bacc.Bacc("TRN2", target_bir_lowering=False, debug=False)
    ins = [
        nc.dram_tensor("xk", [3 * K], f32, kind="ExternalInput").ap(),
        nc.dram_tensor("rows", [K, 128], f32, kind="ExternalInput").ap(),
        nc.dram_tensor("qb", [NQ_CORE, 6], f32, kind="ExternalInput").ap(),
        nc.dram_tensor("qbT3", [3, RB], f32, kind="ExternalInput").ap(),
    ]
    outs = [
        nc.dram_tensor("nf", [3 + C, NQ_CORE, NS], f32, kind="ExternalOutput").ap(),
        nc.dram_tensor("counts", [NQ_CORE, 1], f32, kind="ExternalOutput").ap(),
    ]
    with tile.TileContext(nc) as tc:
        _build(tc, outs, ins)
    nc.compile()
    _CACHE["nc"] = nc
    return nc


def _in_maps(kx, kf, qbox):
    per_b = {}
    for b in range(B):
        rows = np.zeros((K, 128), np.float32)
        rows[:, 0:3] = kx[b, :K, :]
        rows[:, 16:16 + C] = kf[b, :, :K].T
        per_b[b] = dict(
            xk=np.ascontiguousarray(kx[b, :K, :].reshape(-1)),
            rows=rows,
        )
    maps = []
    for core in range(N_CORES):
        b, half = core // 2, core % 2
        qs = np.ascontiguousarray(qbox[b, half * NQ_CORE:(half + 1) * NQ_CORE, :])
        m = dict(per_b[b])
        m.update(
            qb=qs,
            qbT3=np.ascontiguousarray(
                np.repeat(qs[:, :3].T[:, :, None], 8, axis=2).reshape(3, -1)),
        )
        maps.append(m)
    return maps


def _host_fix_query(kx_b, kf_b, qbox_bq, nf_bq, gx_bq):
    """Exact recompute of one (b, q) pair on the host (window fallback)."""
    center, size = qbox_bq[:3], qbox_bq[3:]
    off = np.abs(kx_b - center[None, :])
    inb = (off <= size[None, :] * 0.5).all(-1)
    z = ~inb
    z[0] = True
    zi = np.flatnonzero(z)[:NS]
    gxq = kx_b[zi, :].T - center[:, None]
    gx_bq[:] = gxq
    nf_bq[0:3] = gxq
    nf_bq[3:] = kf_b[:, zi]


def kernel(key_xyz, key_features, query_box, _want_timing=False):
    kx = np.ascontiguousarray(np.asarray(key_xyz, dtype=np.float32))
    kf = np.ascontiguousarray(np.asarray(key_features, dtype=np.float32))
    qbox = np.ascontiguousarray(np.asarray(query_box, dtype=np.float32))
    assert kx.shape == (B, N, 3) and kf.shape == (B, C, N) and qbox.shape == (B, NQ, 6)

    nc = _get_program()
    res = run_bass_kernel_spmd(nc, _in_maps(kx, kf, qbox), list(range(N_CORES)))

    new_features = np.empty((B, 3 + C, NQ, NS), np.float32)
    mask = np.zeros((B, NQ, NS), dtype=bool)
    for core in range(N_CORES):
        b, half = core // 2, core % 2
        sl = slice(half * NQ_CORE, (half + 1) * NQ_CORE)
        r = res.results[core]
        new_features[b, :, sl, :] = r["nf"]
        flag = r["counts"][:, 0]
        if (flag <= 0).any():
            for q in np.flatnonzero(flag <= 0):
                gq = half * NQ_CORE + int(q)
                gx_scratch = np.empty((3, NS), np.float32)
                _host_fix_query(
                    kx[b], kf[b], qbox[b, gq],
                    new_features[b, :, gq, :], gx_scratch,
                )
    grouped_xyz = np.ascontiguousarray(new_features[:, 0:3])
    out = (grouped_xyz, new_features, mask)
    if _want_timing:
        return out, res
    return out
